# revision 8
# baseline (speedup 1.0000x reference)
"""3-layer GCN (message passing) on 8 TRN2 NeuronCores.

Strategy: shard destination nodes across cores (graph parallel). Each layer:
  h'_T = prev @ W  computed locally on the node shard (PE),
  AllGather h' rows (bf16) so every core sees all source features,
  per (dst block, lo/hi half): dma_gather source rows on 4 parallel SWDGE
  queues (edges sorted by dst block, lo/hi split so gather indices fit
  int16; trailing pad slots use idx=-1 which the Q7 desc-gen skips),
  S_w[e,d] = (dstloc[e]==d)*norm[e] built ON-CHIP by DVE in layer 1 and
  cached in DRAM for layers 2-3, and out_T = M.T @ S_w accumulates on the
  PE into a group-wide PSUM bank. Epilogue: out = pg + selfw*h' + bias,
  LeakyReLU. GCN symmetric normalization (including self loops) is folded
  into per-edge weights (WCOL) and self weights (SELFWB) on the host, so
  there is no on-device degree pass.
"""

import numpy as np

import concourse.bacc as bacc
import concourse.mybir as mybir
from concourse.tile import TileContext
from concourse.bass_utils import run_bass_kernel_spmd

try:
    import ml_dtypes

    BF16 = ml_dtypes.bfloat16
except ImportError:  # pragma: no cover
    BF16 = None

N_CORES = 8
D = 128
NEG_SLOPE = 0.1
G_BLOCKS = 4  # dst blocks per gather call group
LEAKY_VIA_PRELU = True
MT_BUFS = 3


def _ceil_div(a, b):
    return (a + b - 1) // b


def _wrap_idx(idx):
    """[cnt] int16 -> [128, cnt//16] wrapped layout (16-partition, replicated x8)."""
    cnt = idx.shape[0]
    assert cnt % 16 == 0
    w = idx.reshape(cnt // 16, 16).T  # [16, cnt//16]
    return np.tile(w, (8, 1)).astype(np.int16)  # [128, cnt//16]


def _preprocess(x, edge_index, edge_attr, edge_type, edge_type_scale):
    """Host-side sharding/layout. Returns (meta, per-core input arrays)."""
    Nn = x.shape[0]
    assert Nn % N_CORES == 0
    per = Nn // N_CORES
    nb = _ceil_div(per, 128)
    per_pad = nb * 128
    # split each core's shard rows at SA: half A rows [0,SA), half B [SA,per).
    SA = max(16, ((per // 2) // 16) * 16)
    SB = per - SA
    assert SA * N_CORES <= 32767 + 1 and SB * N_CORES <= 32767 + 1

    src_f = np.asarray(edge_index[0], dtype=np.int64)
    dst_f = np.asarray(edge_index[1], dtype=np.int64)
    w = np.asarray(edge_type_scale, np.float32)[
        np.asarray(edge_type, np.int64)
    ] * np.asarray(edge_attr, np.float32)
    # symmetric GCN norm with self loops, computed on host
    deg = np.bincount(dst_f, weights=w, minlength=Nn).astype(np.float32) + 1.0
    dinv = 1.0 / np.sqrt(deg)
    norm = (dinv[src_f] * w * dinv[dst_f]).astype(np.float32)
    selfw = (dinv * dinv).astype(np.float32)

    core = dst_f // per
    ldst = dst_f - core * per
    blk = ldst >> 7
    slot = ldst & 127
    src_c = src_f // per
    src_r = src_f - src_c * per
    half = (src_r >= SA).astype(np.int64)
    gidx = np.where(half == 0, src_c * SA + src_r, src_c * SB + (src_r - SA))

    counts = np.zeros((N_CORES, nb, 2), dtype=np.int64)
    per_core = []
    for c in range(N_CORES):
        m = core == c
        s_src = src_f[m]
        s_blk = blk[m]
        s_half = half[m]
        order = np.lexsort((s_src, s_half, s_blk))
        per_core.append(
            dict(
                src=gidx[m][order],
                half=s_half[order],
                blk=s_blk[order],
                slot=slot[m][order],
                norm=norm[m][order],
            )
        )
        cnt = np.bincount(s_blk * 2 + s_half, minlength=nb * 2).reshape(nb, 2)
        counts[c] = cnt

    # common padded schedule: tiles per (block, half), maxed over cores
    tiles_bh = np.maximum(1, _ceil_div(counts.max(axis=0), 128))  # [nb, 2]
    pad_bh = tiles_bh * 128

    groups = [list(range(g, min(g + G_BLOCKS, nb))) for g in range(0, nb, G_BLOCKS)]
    slot_off = np.zeros((nb, 2), dtype=np.int64)
    call_cnt = []  # per (g, half): total padded count
    off = 0
    for g in groups:
        for h in (0, 1):
            c0 = off
            for b in g:
                slot_off[b, h] = off
                off += pad_bh[b, h]
            call_cnt.append(off - c0)
    totslot = off
    T = totslot // 128

    tcols_b = []
    for b in range(nb):
        cols = list(range(slot_off[b, 0] // 128, slot_off[b, 0] // 128 + tiles_bh[b, 0]))
        cols += list(range(slot_off[b, 1] // 128, slot_off[b, 1] // 128 + tiles_bh[b, 1]))
        tcols_b.append(cols)

    ins = []
    for c in range(N_CORES):
        pc = per_core[c]
        idx_sl = np.zeros(totslot, dtype=np.int16)  # pads gather row 0 (norm=0)
        dst_sl = np.zeros(totslot, dtype=np.float32)
        wc_sl = np.zeros(totslot, dtype=np.float32)
        e0 = 0
        for b in range(nb):
            for h in (0, 1):
                n = counts[c, b, h]
                o = slot_off[b, h]
                if n:
                    sl = slice(e0, e0 + n)
                    idx_sl[o : o + n] = pc["src"][sl].astype(np.int16)
                    dst_sl[o : o + n] = pc["slot"][sl]
                    wc_sl[o : o + n] = pc["norm"][sl]
                    e0 += n

        wrapped = []
        off2 = 0
        for cc in call_cnt:
            wrapped.append(_wrap_idx(idx_sl[off2 : off2 + cc]))
            off2 += cc
        idx_w = np.concatenate(wrapped, axis=1)  # [128, totslot//16]

        col = lambda a: np.ascontiguousarray(a.reshape(T, 128).T)  # [128, T]
        xt = np.zeros((128, per_pad), dtype=np.float32)
        xt[:, :per] = np.asarray(x[c * per : (c + 1) * per], dtype=np.float32).T
        sw2 = np.zeros((128, per_pad), dtype=np.float32)
        sw2[:, :per] = selfw[c * per : (c + 1) * per][None, :]
        ins.append(
            dict(
                IDX=idx_w,
                DSTLOC=col(dst_sl).astype(BF16),
                WCOL=col(wc_sl).astype(BF16),
                SELFWB=sw2.astype(BF16),
                XT=xt,
            )
        )

    meta = dict(
        N=Nn, per=per, nb=nb, per_pad=per_pad, SA=SA, T=T,
        totslot=totslot, groups=groups, call_cnt=call_cnt, tiles_bh=tiles_bh,
        slot_off=slot_off, tcols_b=tcols_b,
    )
    return meta, ins


def _build(meta):
    per = meta["per"]
    nb = meta["nb"]
    per_pad = meta["per_pad"]
    SA = meta["SA"]
    SB = per - SA
    T = meta["T"]
    totslot = meta["totslot"]
    groups = meta["groups"]
    call_cnt = meta["call_cnt"]
    tiles_bh = meta["tiles_bh"]
    tcols_b = meta["tcols_b"]

    f32 = mybir.dt.float32
    bf16 = mybir.dt.bfloat16
    i16 = mybir.dt.int16

    maxw16 = max(c // 16 for c in call_cnt)
    maxw128 = max(c // 128 for c in call_cnt)
    call_base = [sum(call_cnt[:i]) for i in range(len(call_cnt))]

    nc = bacc.Bacc("TRN2", num_devices=N_CORES, num_swdge_queues=4,
                   dynamic_dma_scratch_size=65536)

    t_idx = nc.dram_tensor("IDX", [128, totslot // 16], i16, kind="ExternalInput")
    t_dstloc = nc.dram_tensor("DSTLOC", [128, T], bf16, kind="ExternalInput")
    t_wcol = nc.dram_tensor("WCOL", [128, T], bf16, kind="ExternalInput")
    t_selfw = nc.dram_tensor("SELFWB", [128, per_pad], bf16, kind="ExternalInput")
    t_xt = nc.dram_tensor("XT", [128, per_pad], f32, kind="ExternalInput")
    t_W = [
        nc.dram_tensor(f"W{i}", [128, 128], f32, kind="ExternalInput") for i in (1, 2, 3)
    ]
    t_b = [
        nc.dram_tensor(f"b{i}", [128, 1], f32, kind="ExternalInput") for i in (1, 2, 3)
    ]
    t_iota_b = nc.dram_tensor("IOTAB", [128, 128], bf16, kind="ExternalInput")
    t_ident = nc.dram_tensor("IDENT", [128, 128], f32, kind="ExternalInput")
    t_identb = nc.dram_tensor("IDENTB", [128, 128], bf16, kind="ExternalInput")
    t_out = nc.dram_tensor("OUT", [per, 128], f32, kind="ExternalOutput")

    hcurA = [
        nc.dram_tensor(f"hcurA{l}", [SA, 128], bf16, kind="Internal") for l in range(3)
    ]
    hcurB = [
        nc.dram_tensor(f"hcurB{l}", [SB, 128], bf16, kind="Internal") for l in range(3)
    ]
    hfullA = [
        nc.dram_tensor(
            f"hfullA{l}", [N_CORES * SA, 128], bf16, kind="Internal",
            addr_space="Shared",
        )
        for l in range(3)
    ]
    hfullB = [
        nc.dram_tensor(
            f"hfullB{l}", [N_CORES * SB, 128], bf16, kind="Internal",
            addr_space="Shared",
        )
        for l in range(3)
    ]
    t_swc = nc.dram_tensor("swcache", [totslot, 128], bf16, kind="Internal")
    rg = [list(range(N_CORES))]

    def chunks512(total):
        out = []
        o = 0
        while o < total:
            w = min(512, total - o)
            out.append((o, w))
            o += w
        return out

    with TileContext(nc) as tc:
        with (
            tc.tile_pool(name="persist", bufs=1) as pp,
            tc.tile_pool(name="work", bufs=2) as wp,
            tc.tile_pool(name="swp", bufs=3) as swp,
            tc.tile_pool(name="psum", bufs=2, space="PSUM") as psp,
            tc.tile_pool(name="psumg", bufs=3, space="PSUM") as pspg,
        ):
            # ---------- persistent loads ----------
            DSTLOC = pp.tile([128, T], bf16, tag="DSTLOC")
            nc.sync.dma_start(DSTLOC[:, :], t_dstloc[:, :])
            WCOL = pp.tile([128, T], bf16, tag="WCOL")
            nc.sync.dma_start(WCOL[:, :], t_wcol[:, :])
            SELFWB = pp.tile([128, per_pad], bf16, tag="SELFWB")
            nc.sync.dma_start(SELFWB[:, :], t_selfw[:, :])
            IOTAB = pp.tile([128, 128], bf16, tag="IOTAB")
            nc.sync.dma_start(IOTAB[:, :], t_iota_b[:, :])
            IDENT = pp.tile([128, 128], f32, tag="IDENT")
            nc.sync.dma_start(IDENT[:, :], t_ident[:, :])
            IDENTB = pp.tile([128, 128], bf16, tag="IDENTB")
            nc.sync.dma_start(IDENTB[:, :], t_identb[:, :])
            W = []
            B = []
            for i in range(3):
                Wt = pp.tile([128, 128], f32, tag=f"W{i}")
                nc.sync.dma_start(Wt[:, :], t_W[i][:, :])
                W.append(Wt)
                Bt = pp.tile([128, 1], f32, tag=f"B{i}")
                nc.sync.dma_start(Bt[:, :], t_b[i][:, :])
                B.append(Bt)

            HOUT = pp.tile([128, per_pad], f32, tag="HOUT")
            HP = pp.tile([128, per_pad], bf16, tag="HP")

            # ---------- h1' = x @ W1 ----------
            for o, cw in chunks512(per_pad):
                xc = wp.tile([128, 512], f32, tag="xc")
                nc.sync.dma_start(xc[:, :cw], t_xt[:, o : o + cw])
                ph = psp.tile([128, 512], f32, tag="p512")
                nc.tensor.matmul(ph[:, :cw], W[0][:, :], xc[:, :cw], start=True, stop=True)
                nc.vector.tensor_copy(HP[:, o : o + cw], ph[:, :cw])

            # ---------- layers ----------
            for l in range(3):
                # HP (feat x node, bf16) -> rows (PE transpose) -> hcurA/B
                # -> two AllGathers (A fires as soon as its rows are stored)
                ag_a_done = False
                for cb in range(nb):
                    pt = psp.tile([128, 128], bf16, tag="ptr", bufs=1)
                    nc.tensor.transpose(
                        pt[:, :], HP[:, cb * 128 : (cb + 1) * 128], IDENTB[:, :]
                    )
                    rt = wp.tile([128, 128], bf16, tag="rowb")
                    nc.vector.tensor_copy(rt[:, :], pt[:, :])
                    r0 = cb * 128
                    r1 = min(per, r0 + 128)
                    if r1 <= SA:
                        nc.sync.dma_start(hcurA[l][r0:r1, :], rt[0 : r1 - r0, :])
                    elif r0 >= SA:
                        nc.sync.dma_start(
                            hcurB[l][r0 - SA : r1 - SA, :], rt[0 : r1 - r0, :]
                        )
                    else:
                        nc.sync.dma_start(hcurA[l][r0:SA, :], rt[0 : SA - r0, :])
                        nc.sync.dma_start(
                            hcurB[l][0 : r1 - SA, :], rt[SA - r0 : r1 - r0, :]
                        )
                    if r1 >= SA and not ag_a_done:
                        nc.gpsimd.collective_compute(
                            "AllGather", mybir.AluOpType.bypass,
                            ins=[hcurA[l][:, :]], outs=[hfullA[l][:, :]],
                            replica_groups=rg,
                        )
                        ag_a_done = True
                nc.gpsimd.collective_compute(
                    "AllGather", mybir.AluOpType.bypass,
                    ins=[hcurB[l][:, :]], outs=[hfullB[l][:, :]],
                    replica_groups=rg,
                )

                for gi, g in enumerate(groups):
                    gw = len(g) * 128
                    mts = {}
                    for h in (0, 1):
                        ci = 2 * gi + h
                        cnt = call_cnt[ci]
                        woff = call_base[ci] // 16
                        idxt = wp.tile([128, maxw16], i16, tag="idx", bufs=8)
                        nc.sync.dma_start(
                            idxt[:, : cnt // 16], t_idx[:, woff : woff + cnt // 16]
                        )
                        mt = wp.tile([128, maxw128, 128], bf16, tag="mtile",
                                     bufs=MT_BUFS)
                        src_tab = hfullA[l][:, :] if h == 0 else hfullB[l][:, :]
                        nc.gpsimd.dma_gather(
                            mt[:, : cnt // 128, :], src_tab, idxt[:, : cnt // 16],
                            num_idxs=cnt, num_idxs_reg=cnt, elem_size=128,
                            single_packet=False, queue_num=ci % 4,
                        )
                        mts[h] = mt
                    sws = {}
                    for h in (0, 1):
                        ci2 = 2 * gi + h
                        cnt2 = call_cnt[ci2]
                        nt2 = cnt2 // 128
                        base2 = call_base[ci2] // 128
                        swl = swp.tile([128, maxw128, 128], bf16, tag="swg")
                        if l == 0:
                            # build S_w on-chip: (dstloc==iota) * norm
                            dl_b = DSTLOC[:, base2 : base2 + nt2].unsqueeze(
                                2
                            ).to_broadcast([128, nt2, 128])
                            io_b = IOTAB[:, :].unsqueeze(1).to_broadcast(
                                [128, nt2, 128]
                            )
                            nc.vector.tensor_tensor(
                                swl[:, :nt2, :], dl_b, io_b,
                                op=mybir.AluOpType.is_equal,
                            )
                            wc_b = WCOL[:, base2 : base2 + nt2].unsqueeze(
                                2
                            ).to_broadcast([128, nt2, 128])
                            nc.vector.tensor_tensor(
                                swl[:, :nt2, :], swl[:, :nt2, :], wc_b,
                                op=mybir.AluOpType.mult,
                            )
                            swv = t_swc[
                                call_base[ci2] : call_base[ci2] + cnt2, :
                            ].rearrange("(t e) d -> e t d", e=128)
                            nc.sync.dma_start(swv, swl[:, :nt2, :])
                        else:
                            swv = t_swc[
                                call_base[ci2] : call_base[ci2] + cnt2, :
                            ].rearrange("(t e) d -> e t d", e=128)
                            nc.sync.dma_start(swl[:, :nt2, :], swv)
                        sws[h] = swl

                    pg = pspg.tile([128, 512], f32, tag="pblk")
                    for bi, b in enumerate(g):
                        cols = tcols_b[b]
                        nlo = int(tiles_bh[b][0])
                        for ti, tcol in enumerate(cols):
                            hh = 0 if ti < nlo else 1
                            j = tcol - call_base[2 * gi + hh] // 128
                            nc.tensor.matmul(
                                pg[:, bi * 128 : (bi + 1) * 128],
                                mts[hh][:, j, :], sws[hh][:, j, :],
                                start=(ti == 0), stop=(ti == len(cols) - 1),
                            )
                    # batched epilogue for the whole group:
                    # out = pg + selfw*h' (+bias, leaky via Prelu)
                    g0 = g[0] * 128
                    ep = wp.tile([128, 512], f32, tag="ep")
                    nc.vector.tensor_tensor(
                        ep[:, :gw], SELFWB[:, g0 : g0 + gw], HP[:, g0 : g0 + gw],
                        op=mybir.AluOpType.mult,
                    )
                    nc.vector.tensor_tensor(
                        ep[:, :gw], ep[:, :gw], pg[:, :gw],
                        op=mybir.AluOpType.add,
                    )
                    if l < 2 and LEAKY_VIA_PRELU:
                        nc.scalar.activation(
                            HOUT[:, g0 : g0 + gw], ep[:, :gw],
                            mybir.ActivationFunctionType.Prelu,
                            bias=B[l][:, 0:1], scale=1.0, alpha=NEG_SLOPE,
                        )
                    elif l < 2:
                        t2 = wp.tile([128, 512], f32, tag="ep2")
                        nc.scalar.activation(
                            t2[:, :gw], ep[:, :gw],
                            mybir.ActivationFunctionType.Identity,
                            bias=B[l][:, 0:1], scale=1.0,
                        )
                        t3 = wp.tile([128, 512], f32, tag="ep3")
                        nc.vector.tensor_scalar_mul(t3[:, :gw], t2[:, :gw], NEG_SLOPE)
                        nc.vector.tensor_tensor(
                            HOUT[:, g0 : g0 + gw], t2[:, :gw], t3[:, :gw],
                            op=mybir.AluOpType.max,
                        )
                    else:
                        nc.scalar.activation(
                            HOUT[:, g0 : g0 + gw], ep[:, :gw],
                            mybir.ActivationFunctionType.Identity,
                            bias=B[l][:, 0:1], scale=1.0,
                        )

                if l < 2:
                    # HP = HOUT @ W[l+1]
                    for o, cw in chunks512(per_pad):
                        ph = psp.tile([128, 512], f32, tag="p512")
                        nc.tensor.matmul(
                            ph[:, :cw], W[l + 1][:, :], HOUT[:, o : o + cw],
                            start=True, stop=True,
                        )
                        nc.vector.tensor_copy(HP[:, o : o + cw], ph[:, :cw])
                else:
                    # final: transpose HOUT (f32) to rows and store
                    for cb in range(nb):
                        pt = psp.tile([128, 128], f32, tag="ptrf", bufs=1)
                        nc.tensor.transpose(
                            pt[:, :], HOUT[:, cb * 128 : (cb + 1) * 128], IDENT[:, :]
                        )
                        rf = wp.tile([128, 128], f32, tag="rowf")
                        nc.vector.tensor_copy(rf[:, :], pt[:, :])
                        r0 = cb * 128
                        r1 = min(per, r0 + 128)
                        nc.sync.dma_start(t_out[r0:r1, :], rf[0 : r1 - r0, :])

    nc.compile()
    return nc


_CACHE = {}


def kernel(
    x,
    edge_index,
    edge_attr,
    edge_type,
    edge_type_scale,
    W1,
    b1,
    W2,
    b2,
    W3,
    b3,
):
    x = np.asarray(x)
    Nn = x.shape[0]
    meta, per_core = _preprocess(
        np.asarray(x), np.asarray(edge_index), np.asarray(edge_attr),
        np.asarray(edge_type), np.asarray(edge_type_scale),
    )

    key = (Nn, meta["T"], tuple(meta["call_cnt"]))
    if key not in _CACHE:
        _CACHE[key] = _build(meta)
    nc = _CACHE[key]

    iota_f = np.tile(np.arange(128, dtype=np.float32)[None, :], (128, 1))
    ident = np.eye(128, dtype=np.float32)
    common = dict(
        W1=np.asarray(W1, np.float32),
        W2=np.asarray(W2, np.float32),
        W3=np.asarray(W3, np.float32),
        b1=np.asarray(b1, np.float32).reshape(D, 1),
        b2=np.asarray(b2, np.float32).reshape(D, 1),
        b3=np.asarray(b3, np.float32).reshape(D, 1),
        IOTAB=iota_f.astype(BF16),
        IDENT=ident,
        IDENTB=ident.astype(BF16),
    )
    in_maps = []
    for c in range(N_CORES):
        m = dict(common)
        m.update(per_core[c])
        in_maps.append(m)

    res = run_bass_kernel_spmd(
        nc, in_maps, core_ids=list(range(N_CORES)), **_RUN_KWARGS
    )
    _LAST_RESULT.clear()
    _LAST_RESULT["exec_time_ns"] = res.exec_time_ns
    _LAST_RESULT["profile_json"] = res.profile_json
    out = np.concatenate([res.results[c]["OUT"] for c in range(N_CORES)], axis=0)
    return out.astype(np.float32)


_RUN_KWARGS = {}  # test harness can set {"trace": True, "tmpdir": ...}
_LAST_RESULT = {}


# revision 10
# speedup vs baseline: 1.3264x; 1.3264x over previous
"""3-layer GCN (message passing) on 8 TRN2 NeuronCores.

Strategy: shard destination nodes across cores (graph parallel). Each layer:
  h'_T = prev @ W  computed locally on the node shard (PE),
  AllGather h' rows (bf16) so every core sees all source features,
  per (dst block, lo/hi half): dma_gather source rows on 4 parallel SWDGE
  queues (edges sorted by dst block, lo/hi split so gather indices fit
  int16; trailing pad slots use idx=-1 which the Q7 desc-gen skips),
  S_w[e,d] = (dstloc[e]==d)*norm[e] built ON-CHIP by DVE in layer 1 and
  cached in DRAM for layers 2-3, and out_T = M.T @ S_w accumulates on the
  PE into a group-wide PSUM bank. Epilogue: out = pg + selfw*h' + bias,
  LeakyReLU. GCN symmetric normalization (including self loops) is folded
  into per-edge weights (WCOL) and self weights (SELFWB) on the host, so
  there is no on-device degree pass.
"""

import numpy as np

import concourse.bacc as bacc
import concourse.mybir as mybir
from concourse.tile import TileContext
from concourse.bass_utils import run_bass_kernel_spmd

try:
    import ml_dtypes

    BF16 = ml_dtypes.bfloat16
except ImportError:  # pragma: no cover
    BF16 = None

N_CORES = 8
D = 128
NEG_SLOPE = 0.1
G_BLOCKS = 1  # dst blocks per gather call group
LEAKY_VIA_PRELU = True
MT_BUFS = 8


def _ceil_div(a, b):
    return (a + b - 1) // b


def _wrap_idx(idx):
    """[cnt] int16 -> [128, cnt//16] wrapped layout (16-partition, replicated x8)."""
    cnt = idx.shape[0]
    assert cnt % 16 == 0
    w = idx.reshape(cnt // 16, 16).T  # [16, cnt//16]
    return np.tile(w, (8, 1)).astype(np.int16)  # [128, cnt//16]


def _preprocess(x, edge_index, edge_attr, edge_type, edge_type_scale):
    """Host-side sharding/layout. Returns (meta, per-core input arrays)."""
    Nn = x.shape[0]
    assert Nn % N_CORES == 0
    per = Nn // N_CORES
    nb = _ceil_div(per, 128)
    per_pad = nb * 128
    # split each core's shard rows at SA: half A rows [0,SA), half B [SA,per).
    SA = max(16, ((per // 2) // 16) * 16)
    SB = per - SA
    assert SA * N_CORES <= 32767 + 1 and SB * N_CORES <= 32767 + 1

    src_f = np.asarray(edge_index[0], dtype=np.int64)
    dst_f = np.asarray(edge_index[1], dtype=np.int64)
    w = np.asarray(edge_type_scale, np.float32)[
        np.asarray(edge_type, np.int64)
    ] * np.asarray(edge_attr, np.float32)
    # symmetric GCN norm with self loops, computed on host
    deg = np.bincount(dst_f, weights=w, minlength=Nn).astype(np.float32) + 1.0
    dinv = 1.0 / np.sqrt(deg)
    norm = (dinv[src_f] * w * dinv[dst_f]).astype(np.float32)
    selfw = (dinv * dinv).astype(np.float32)

    core = dst_f // per
    ldst = dst_f - core * per
    blk = ldst >> 7
    slot = ldst & 127
    src_c = src_f // per
    src_r = src_f - src_c * per
    half = (src_r >= SA).astype(np.int64)
    gidx = np.where(half == 0, src_c * SA + src_r, src_c * SB + (src_r - SA))

    counts = np.zeros((N_CORES, nb, 2), dtype=np.int64)
    per_core = []
    for c in range(N_CORES):
        m = core == c
        s_src = src_f[m]
        s_blk = blk[m]
        s_half = half[m]
        order = np.lexsort((s_src, s_half, s_blk))
        per_core.append(
            dict(
                src=gidx[m][order],
                half=s_half[order],
                blk=s_blk[order],
                slot=slot[m][order],
                norm=norm[m][order],
            )
        )
        cnt = np.bincount(s_blk * 2 + s_half, minlength=nb * 2).reshape(nb, 2)
        counts[c] = cnt

    # common padded schedule: tiles per (block, half), maxed over cores
    tiles_bh = np.maximum(1, _ceil_div(counts.max(axis=0), 128))  # [nb, 2]
    pad_bh = tiles_bh * 128

    groups = [list(range(g, min(g + G_BLOCKS, nb))) for g in range(0, nb, G_BLOCKS)]
    slot_off = np.zeros((nb, 2), dtype=np.int64)
    call_cnt = []  # per (g, half): total padded count
    off = 0
    for g in groups:
        for h in (0, 1):
            c0 = off
            for b in g:
                slot_off[b, h] = off
                off += pad_bh[b, h]
            call_cnt.append(off - c0)
    totslot = off
    T = totslot // 128

    tcols_b = []
    for b in range(nb):
        cols = list(range(slot_off[b, 0] // 128, slot_off[b, 0] // 128 + tiles_bh[b, 0]))
        cols += list(range(slot_off[b, 1] // 128, slot_off[b, 1] // 128 + tiles_bh[b, 1]))
        tcols_b.append(cols)

    ins = []
    for c in range(N_CORES):
        pc = per_core[c]
        idx_sl = np.zeros(totslot, dtype=np.int16)  # pads gather row 0 (norm=0)
        dst_sl = np.zeros(totslot, dtype=np.float32)
        wc_sl = np.zeros(totslot, dtype=np.float32)
        e0 = 0
        for b in range(nb):
            for h in (0, 1):
                n = counts[c, b, h]
                o = slot_off[b, h]
                if n:
                    sl = slice(e0, e0 + n)
                    idx_sl[o : o + n] = pc["src"][sl].astype(np.int16)
                    dst_sl[o : o + n] = pc["slot"][sl]
                    wc_sl[o : o + n] = pc["norm"][sl]
                    e0 += n

        wrapped = []
        off2 = 0
        for cc in call_cnt:
            wrapped.append(_wrap_idx(idx_sl[off2 : off2 + cc]))
            off2 += cc
        idx_w = np.concatenate(wrapped, axis=1)  # [128, totslot//16]

        col = lambda a: np.ascontiguousarray(a.reshape(T, 128).T)  # [128, T]
        xt = np.zeros((128, per_pad), dtype=np.float32)
        xt[:, :per] = np.asarray(x[c * per : (c + 1) * per], dtype=np.float32).T
        sw2 = np.zeros((128, per_pad), dtype=np.float32)
        sw2[:, :per] = selfw[c * per : (c + 1) * per][None, :]
        ins.append(
            dict(
                IDX=idx_w,
                DSTLOC=col(dst_sl).astype(BF16),
                WCOL=col(wc_sl).astype(BF16),
                SELFWB=sw2.astype(BF16),
                XT=xt,
            )
        )

    meta = dict(
        N=Nn, per=per, nb=nb, per_pad=per_pad, SA=SA, T=T,
        totslot=totslot, groups=groups, call_cnt=call_cnt, tiles_bh=tiles_bh,
        slot_off=slot_off, tcols_b=tcols_b,
    )
    return meta, ins


def _build(meta):
    per = meta["per"]
    nb = meta["nb"]
    per_pad = meta["per_pad"]
    SA = meta["SA"]
    SB = per - SA
    T = meta["T"]
    totslot = meta["totslot"]
    groups = meta["groups"]
    call_cnt = meta["call_cnt"]
    tiles_bh = meta["tiles_bh"]
    tcols_b = meta["tcols_b"]

    f32 = mybir.dt.float32
    bf16 = mybir.dt.bfloat16
    i16 = mybir.dt.int16

    maxw16 = max(c // 16 for c in call_cnt)
    maxw128 = max(c // 128 for c in call_cnt)
    call_base = [sum(call_cnt[:i]) for i in range(len(call_cnt))]

    nc = bacc.Bacc("TRN2", num_devices=N_CORES, num_swdge_queues=4,
                   dynamic_dma_scratch_size=65536)

    t_idx = nc.dram_tensor("IDX", [128, totslot // 16], i16, kind="ExternalInput")
    t_dstloc = nc.dram_tensor("DSTLOC", [128, T], bf16, kind="ExternalInput")
    t_wcol = nc.dram_tensor("WCOL", [128, T], bf16, kind="ExternalInput")
    t_selfw = nc.dram_tensor("SELFWB", [128, per_pad], bf16, kind="ExternalInput")
    t_xt = nc.dram_tensor("XT", [128, per_pad], f32, kind="ExternalInput")
    t_W = [
        nc.dram_tensor(f"W{i}", [128, 128], f32, kind="ExternalInput") for i in (1, 2, 3)
    ]
    t_b = [
        nc.dram_tensor(f"b{i}", [128, 1], f32, kind="ExternalInput") for i in (1, 2, 3)
    ]
    t_iota_b = nc.dram_tensor("IOTAB", [128, 128], bf16, kind="ExternalInput")
    t_ident = nc.dram_tensor("IDENT", [128, 128], f32, kind="ExternalInput")
    t_identb = nc.dram_tensor("IDENTB", [128, 128], bf16, kind="ExternalInput")
    t_out = nc.dram_tensor("OUT", [per, 128], f32, kind="ExternalOutput")

    hcurA = [
        nc.dram_tensor(f"hcurA{l}", [SA, 128], bf16, kind="Internal") for l in range(3)
    ]
    hcurB = [
        nc.dram_tensor(f"hcurB{l}", [SB, 128], bf16, kind="Internal") for l in range(3)
    ]
    hfullA = [
        nc.dram_tensor(
            f"hfullA{l}", [N_CORES * SA, 128], bf16, kind="Internal",
            addr_space="Shared",
        )
        for l in range(3)
    ]
    hfullB = [
        nc.dram_tensor(
            f"hfullB{l}", [N_CORES * SB, 128], bf16, kind="Internal",
            addr_space="Shared",
        )
        for l in range(3)
    ]
    t_swc = nc.dram_tensor("swcache", [totslot, 128], bf16, kind="Internal")
    rg = [list(range(N_CORES))]

    def chunks512(total):
        out = []
        o = 0
        while o < total:
            w = min(512, total - o)
            out.append((o, w))
            o += w
        return out

    with TileContext(nc) as tc:
        with (
            tc.tile_pool(name="persist", bufs=1) as pp,
            tc.tile_pool(name="work", bufs=2) as wp,
            tc.tile_pool(name="swp", bufs=4) as swp,
            tc.tile_pool(name="psum", bufs=2, space="PSUM") as psp,
            tc.tile_pool(name="psumg", bufs=3, space="PSUM") as pspg,
        ):
            # ---------- persistent loads ----------
            DSTLOC = pp.tile([128, T], bf16, tag="DSTLOC")
            nc.sync.dma_start(DSTLOC[:, :], t_dstloc[:, :])
            WCOL = pp.tile([128, T], bf16, tag="WCOL")
            nc.sync.dma_start(WCOL[:, :], t_wcol[:, :])
            SELFWB = pp.tile([128, per_pad], bf16, tag="SELFWB")
            nc.sync.dma_start(SELFWB[:, :], t_selfw[:, :])
            IOTAB = pp.tile([128, 128], bf16, tag="IOTAB")
            nc.sync.dma_start(IOTAB[:, :], t_iota_b[:, :])
            IDENT = pp.tile([128, 128], f32, tag="IDENT")
            nc.sync.dma_start(IDENT[:, :], t_ident[:, :])
            IDENTB = pp.tile([128, 128], bf16, tag="IDENTB")
            nc.sync.dma_start(IDENTB[:, :], t_identb[:, :])
            W = []
            B = []
            for i in range(3):
                Wt = pp.tile([128, 128], f32, tag=f"W{i}")
                nc.sync.dma_start(Wt[:, :], t_W[i][:, :])
                W.append(Wt)
                Bt = pp.tile([128, 1], f32, tag=f"B{i}")
                nc.sync.dma_start(Bt[:, :], t_b[i][:, :])
                B.append(Bt)

            HOUT = pp.tile([128, per_pad], f32, tag="HOUT")
            HP = pp.tile([128, per_pad], bf16, tag="HP")

            # ---------- h1' = x @ W1 ----------
            for o, cw in chunks512(per_pad):
                xc = wp.tile([128, 512], f32, tag="xc")
                nc.sync.dma_start(xc[:, :cw], t_xt[:, o : o + cw])
                ph = psp.tile([128, 512], f32, tag="p512")
                nc.tensor.matmul(ph[:, :cw], W[0][:, :], xc[:, :cw], start=True, stop=True)
                nc.vector.tensor_copy(HP[:, o : o + cw], ph[:, :cw])

            # ---------- layers ----------
            for l in range(3):
                # HP (feat x node, bf16) -> rows (PE transpose) -> hcurA/B
                # -> two AllGathers (A fires as soon as its rows are stored)
                ag_a_done = False
                for cb in range(nb):
                    pt = psp.tile([128, 128], bf16, tag="ptr", bufs=1)
                    nc.tensor.transpose(
                        pt[:, :], HP[:, cb * 128 : (cb + 1) * 128], IDENTB[:, :]
                    )
                    rt = wp.tile([128, 128], bf16, tag="rowb")
                    nc.vector.tensor_copy(rt[:, :], pt[:, :])
                    r0 = cb * 128
                    r1 = min(per, r0 + 128)
                    if r1 <= SA:
                        nc.sync.dma_start(hcurA[l][r0:r1, :], rt[0 : r1 - r0, :])
                    elif r0 >= SA:
                        nc.sync.dma_start(
                            hcurB[l][r0 - SA : r1 - SA, :], rt[0 : r1 - r0, :]
                        )
                    else:
                        nc.sync.dma_start(hcurA[l][r0:SA, :], rt[0 : SA - r0, :])
                        nc.sync.dma_start(
                            hcurB[l][0 : r1 - SA, :], rt[SA - r0 : r1 - r0, :]
                        )
                    if r1 >= SA and not ag_a_done:
                        nc.gpsimd.collective_compute(
                            "AllGather", mybir.AluOpType.bypass,
                            ins=[hcurA[l][:, :]], outs=[hfullA[l][:, :]],
                            replica_groups=rg,
                        )
                        ag_a_done = True
                nc.gpsimd.collective_compute(
                    "AllGather", mybir.AluOpType.bypass,
                    ins=[hcurB[l][:, :]], outs=[hfullB[l][:, :]],
                    replica_groups=rg,
                )

                for gi, g in enumerate(groups):
                    gw = len(g) * 128
                    mts = {}
                    for h in (0, 1):
                        ci = 2 * gi + h
                        cnt = call_cnt[ci]
                        woff = call_base[ci] // 16
                        idxt = wp.tile([128, maxw16], i16, tag="idx", bufs=16)
                        nc.sync.dma_start(
                            idxt[:, : cnt // 16], t_idx[:, woff : woff + cnt // 16]
                        )
                        mt = wp.tile([128, maxw128, 128], bf16, tag="mtile",
                                     bufs=MT_BUFS)
                        src_tab = hfullA[l][:, :] if h == 0 else hfullB[l][:, :]
                        nc.gpsimd.dma_gather(
                            mt[:, : cnt // 128, :], src_tab, idxt[:, : cnt // 16],
                            num_idxs=cnt, num_idxs_reg=cnt, elem_size=128,
                            single_packet=False, queue_num=ci % 4,
                        )
                        mts[h] = mt
                    sws = {}
                    for h in (0, 1):
                        ci2 = 2 * gi + h
                        cnt2 = call_cnt[ci2]
                        nt2 = cnt2 // 128
                        base2 = call_base[ci2] // 128
                        swl = swp.tile([128, maxw128, 128], bf16, tag="swg")
                        if l == 0:
                            # build S_w on-chip: (dstloc==iota) * norm
                            dl_b = DSTLOC[:, base2 : base2 + nt2].unsqueeze(
                                2
                            ).to_broadcast([128, nt2, 128])
                            io_b = IOTAB[:, :].unsqueeze(1).to_broadcast(
                                [128, nt2, 128]
                            )
                            nc.vector.tensor_tensor(
                                swl[:, :nt2, :], dl_b, io_b,
                                op=mybir.AluOpType.is_equal,
                            )
                            wc_b = WCOL[:, base2 : base2 + nt2].unsqueeze(
                                2
                            ).to_broadcast([128, nt2, 128])
                            nc.vector.tensor_tensor(
                                swl[:, :nt2, :], swl[:, :nt2, :], wc_b,
                                op=mybir.AluOpType.mult,
                            )
                            swv = t_swc[
                                call_base[ci2] : call_base[ci2] + cnt2, :
                            ].rearrange("(t e) d -> e t d", e=128)
                            nc.sync.dma_start(swv, swl[:, :nt2, :])
                        else:
                            swv = t_swc[
                                call_base[ci2] : call_base[ci2] + cnt2, :
                            ].rearrange("(t e) d -> e t d", e=128)
                            nc.sync.dma_start(swl[:, :nt2, :], swv)
                        sws[h] = swl

                    pg = pspg.tile([128, 512], f32, tag="pblk")
                    for bi, b in enumerate(g):
                        cols = tcols_b[b]
                        nlo = int(tiles_bh[b][0])
                        for ti, tcol in enumerate(cols):
                            hh = 0 if ti < nlo else 1
                            j = tcol - call_base[2 * gi + hh] // 128
                            nc.tensor.matmul(
                                pg[:, bi * 128 : (bi + 1) * 128],
                                mts[hh][:, j, :], sws[hh][:, j, :],
                                start=(ti == 0), stop=(ti == len(cols) - 1),
                            )
                    # batched epilogue for the whole group:
                    # out = pg + selfw*h' (+bias, leaky via Prelu)
                    g0 = g[0] * 128
                    ep = wp.tile([128, 512], f32, tag="ep")
                    nc.vector.tensor_tensor(
                        ep[:, :gw], SELFWB[:, g0 : g0 + gw], HP[:, g0 : g0 + gw],
                        op=mybir.AluOpType.mult,
                    )
                    nc.vector.tensor_tensor(
                        ep[:, :gw], ep[:, :gw], pg[:, :gw],
                        op=mybir.AluOpType.add,
                    )
                    if l < 2 and LEAKY_VIA_PRELU:
                        nc.scalar.activation(
                            HOUT[:, g0 : g0 + gw], ep[:, :gw],
                            mybir.ActivationFunctionType.Prelu,
                            bias=B[l][:, 0:1], scale=1.0, alpha=NEG_SLOPE,
                        )
                    elif l < 2:
                        t2 = wp.tile([128, 512], f32, tag="ep2")
                        nc.scalar.activation(
                            t2[:, :gw], ep[:, :gw],
                            mybir.ActivationFunctionType.Identity,
                            bias=B[l][:, 0:1], scale=1.0,
                        )
                        t3 = wp.tile([128, 512], f32, tag="ep3")
                        nc.vector.tensor_scalar_mul(t3[:, :gw], t2[:, :gw], NEG_SLOPE)
                        nc.vector.tensor_tensor(
                            HOUT[:, g0 : g0 + gw], t2[:, :gw], t3[:, :gw],
                            op=mybir.AluOpType.max,
                        )
                    else:
                        nc.scalar.activation(
                            HOUT[:, g0 : g0 + gw], ep[:, :gw],
                            mybir.ActivationFunctionType.Identity,
                            bias=B[l][:, 0:1], scale=1.0,
                        )

                if l < 2:
                    # HP = HOUT @ W[l+1]
                    for o, cw in chunks512(per_pad):
                        ph = psp.tile([128, 512], f32, tag="p512")
                        nc.tensor.matmul(
                            ph[:, :cw], W[l + 1][:, :], HOUT[:, o : o + cw],
                            start=True, stop=True,
                        )
                        nc.vector.tensor_copy(HP[:, o : o + cw], ph[:, :cw])
                else:
                    # final: transpose HOUT (f32) to rows and store
                    for cb in range(nb):
                        pt = psp.tile([128, 128], f32, tag="ptrf", bufs=1)
                        nc.tensor.transpose(
                            pt[:, :], HOUT[:, cb * 128 : (cb + 1) * 128], IDENT[:, :]
                        )
                        rf = wp.tile([128, 128], f32, tag="rowf")
                        nc.vector.tensor_copy(rf[:, :], pt[:, :])
                        r0 = cb * 128
                        r1 = min(per, r0 + 128)
                        nc.sync.dma_start(t_out[r0:r1, :], rf[0 : r1 - r0, :])

    nc.compile()
    return nc


_CACHE = {}


def kernel(
    x,
    edge_index,
    edge_attr,
    edge_type,
    edge_type_scale,
    W1,
    b1,
    W2,
    b2,
    W3,
    b3,
):
    x = np.asarray(x)
    Nn = x.shape[0]
    meta, per_core = _preprocess(
        np.asarray(x), np.asarray(edge_index), np.asarray(edge_attr),
        np.asarray(edge_type), np.asarray(edge_type_scale),
    )

    key = (Nn, meta["T"], tuple(meta["call_cnt"]))
    if key not in _CACHE:
        _CACHE[key] = _build(meta)
    nc = _CACHE[key]

    iota_f = np.tile(np.arange(128, dtype=np.float32)[None, :], (128, 1))
    ident = np.eye(128, dtype=np.float32)
    common = dict(
        W1=np.asarray(W1, np.float32),
        W2=np.asarray(W2, np.float32),
        W3=np.asarray(W3, np.float32),
        b1=np.asarray(b1, np.float32).reshape(D, 1),
        b2=np.asarray(b2, np.float32).reshape(D, 1),
        b3=np.asarray(b3, np.float32).reshape(D, 1),
        IOTAB=iota_f.astype(BF16),
        IDENT=ident,
        IDENTB=ident.astype(BF16),
    )
    in_maps = []
    for c in range(N_CORES):
        m = dict(common)
        m.update(per_core[c])
        in_maps.append(m)

    res = run_bass_kernel_spmd(
        nc, in_maps, core_ids=list(range(N_CORES)), **_RUN_KWARGS
    )
    _LAST_RESULT.clear()
    _LAST_RESULT["exec_time_ns"] = res.exec_time_ns
    _LAST_RESULT["profile_json"] = res.profile_json
    out = np.concatenate([res.results[c]["OUT"] for c in range(N_CORES)], axis=0)
    return out.astype(np.float32)


_RUN_KWARGS = {}  # test harness can set {"trace": True, "tmpdir": ...}
_LAST_RESULT = {}


# revision 11
# speedup vs baseline: 1.4247x; 1.0741x over previous
"""3-layer GCN (message passing) on 8 TRN2 NeuronCores.

Strategy: shard destination nodes across cores (graph parallel). Each layer:
  h'_T = prev @ W  computed locally on the node shard (PE),
  AllGather h' rows (bf16) so every core sees all source features,
  per (dst block, lo/hi half): dma_gather source rows on 4 parallel SWDGE
  queues (edges sorted by dst block, lo/hi split so gather indices fit
  int16; trailing pad slots use idx=-1 which the Q7 desc-gen skips),
  S_w[e,d] = (dstloc[e]==d)*norm[e] built ON-CHIP by DVE in layer 1 and
  cached in DRAM for layers 2-3, and out_T = M.T @ S_w accumulates on the
  PE into a group-wide PSUM bank. Epilogue: out = pg + selfw*h' + bias,
  LeakyReLU. GCN symmetric normalization (including self loops) is folded
  into per-edge weights (WCOL) and self weights (SELFWB) on the host, so
  there is no on-device degree pass.
"""

import numpy as np

import concourse.bacc as bacc
import concourse.mybir as mybir
from concourse.tile import TileContext
from concourse.bass_utils import run_bass_kernel_spmd

try:
    import ml_dtypes

    BF16 = ml_dtypes.bfloat16
except ImportError:  # pragma: no cover
    BF16 = None

N_CORES = 8
D = 128
NEG_SLOPE = 0.1
G_BLOCKS = 1  # dst blocks per gather call group
LEAKY_VIA_PRELU = True
MT_BUFS = 12


def _ceil_div(a, b):
    return (a + b - 1) // b


def _wrap_idx(idx):
    """[cnt] int16 -> [128, cnt//16] wrapped layout (16-partition, replicated x8)."""
    cnt = idx.shape[0]
    assert cnt % 16 == 0
    w = idx.reshape(cnt // 16, 16).T  # [16, cnt//16]
    return np.tile(w, (8, 1)).astype(np.int16)  # [128, cnt//16]


def _preprocess(x, edge_index, edge_attr, edge_type, edge_type_scale):
    """Host-side sharding/layout. Returns (meta, per-core input arrays)."""
    Nn = x.shape[0]
    assert Nn % N_CORES == 0
    per = Nn // N_CORES
    nb = _ceil_div(per, 128)
    per_pad = nb * 128
    # split each core's shard rows at SA: half A rows [0,SA), half B [SA,per).
    SA = max(16, ((per // 2) // 16) * 16)
    SB = per - SA
    assert SA * N_CORES <= 32767 + 1 and SB * N_CORES <= 32767 + 1

    src_f = np.asarray(edge_index[0], dtype=np.int64)
    dst_f = np.asarray(edge_index[1], dtype=np.int64)
    w = np.asarray(edge_type_scale, np.float32)[
        np.asarray(edge_type, np.int64)
    ] * np.asarray(edge_attr, np.float32)
    # symmetric GCN norm with self loops, computed on host
    deg = np.bincount(dst_f, weights=w, minlength=Nn).astype(np.float32) + 1.0
    dinv = 1.0 / np.sqrt(deg)
    norm = (dinv[src_f] * w * dinv[dst_f]).astype(np.float32)
    selfw = (dinv * dinv).astype(np.float32)

    core = dst_f // per
    ldst = dst_f - core * per
    blk = ldst >> 7
    slot = ldst & 127
    src_c = src_f // per
    src_r = src_f - src_c * per
    half = (src_r >= SA).astype(np.int64)
    gidx = np.where(half == 0, src_c * SA + src_r, src_c * SB + (src_r - SA))

    counts = np.zeros((N_CORES, nb, 2), dtype=np.int64)
    per_core = []
    for c in range(N_CORES):
        m = core == c
        s_src = src_f[m]
        s_blk = blk[m]
        s_half = half[m]
        order = np.lexsort((s_src, s_half, s_blk))
        per_core.append(
            dict(
                src=gidx[m][order],
                half=s_half[order],
                blk=s_blk[order],
                slot=slot[m][order],
                norm=norm[m][order],
            )
        )
        cnt = np.bincount(s_blk * 2 + s_half, minlength=nb * 2).reshape(nb, 2)
        counts[c] = cnt

    # common padded schedule: tiles per (block, half), maxed over cores
    tiles_bh = np.maximum(1, _ceil_div(counts.max(axis=0), 128))  # [nb, 2]
    pad_bh = tiles_bh * 128

    groups = [list(range(g, min(g + G_BLOCKS, nb))) for g in range(0, nb, G_BLOCKS)]
    slot_off = np.zeros((nb, 2), dtype=np.int64)
    call_cnt = []  # per (g, half): total padded count
    off = 0
    for g in groups:
        for h in (0, 1):
            c0 = off
            for b in g:
                slot_off[b, h] = off
                off += pad_bh[b, h]
            call_cnt.append(off - c0)
    totslot = off
    T = totslot // 128

    tcols_b = []
    for b in range(nb):
        cols = list(range(slot_off[b, 0] // 128, slot_off[b, 0] // 128 + tiles_bh[b, 0]))
        cols += list(range(slot_off[b, 1] // 128, slot_off[b, 1] // 128 + tiles_bh[b, 1]))
        tcols_b.append(cols)

    ins = []
    for c in range(N_CORES):
        pc = per_core[c]
        idx_sl = np.zeros(totslot, dtype=np.int16)  # pads gather row 0 (norm=0)
        dst_sl = np.zeros(totslot, dtype=np.float32)
        wc_sl = np.zeros(totslot, dtype=np.float32)
        e0 = 0
        for b in range(nb):
            for h in (0, 1):
                n = counts[c, b, h]
                o = slot_off[b, h]
                if n:
                    sl = slice(e0, e0 + n)
                    idx_sl[o : o + n] = pc["src"][sl].astype(np.int16)
                    dst_sl[o : o + n] = pc["slot"][sl]
                    wc_sl[o : o + n] = pc["norm"][sl]
                    e0 += n

        wrapped = []
        off2 = 0
        for cc in call_cnt:
            wrapped.append(_wrap_idx(idx_sl[off2 : off2 + cc]))
            off2 += cc
        idx_w = np.concatenate(wrapped, axis=1)  # [128, totslot//16]

        col = lambda a: np.ascontiguousarray(a.reshape(T, 128).T)  # [128, T]
        xt = np.zeros((128, per_pad), dtype=np.float32)
        xt[:, :per] = np.asarray(x[c * per : (c + 1) * per], dtype=np.float32).T
        sw2 = np.zeros((128, per_pad), dtype=np.float32)
        sw2[:, :per] = selfw[c * per : (c + 1) * per][None, :]
        ins.append(
            dict(
                IDX=idx_w,
                DSTLOC=col(dst_sl).astype(BF16),
                WCOL=col(wc_sl).astype(BF16),
                SELFWB=sw2.astype(BF16),
                XT=xt,
            )
        )

    meta = dict(
        N=Nn, per=per, nb=nb, per_pad=per_pad, SA=SA, T=T,
        totslot=totslot, groups=groups, call_cnt=call_cnt, tiles_bh=tiles_bh,
        slot_off=slot_off, tcols_b=tcols_b,
    )
    return meta, ins


def _build(meta):
    per = meta["per"]
    nb = meta["nb"]
    per_pad = meta["per_pad"]
    SA = meta["SA"]
    SB = per - SA
    T = meta["T"]
    totslot = meta["totslot"]
    groups = meta["groups"]
    call_cnt = meta["call_cnt"]
    tiles_bh = meta["tiles_bh"]
    tcols_b = meta["tcols_b"]

    f32 = mybir.dt.float32
    bf16 = mybir.dt.bfloat16
    i16 = mybir.dt.int16

    maxw16 = max(c // 16 for c in call_cnt)
    maxw128 = max(c // 128 for c in call_cnt)
    call_base = [sum(call_cnt[:i]) for i in range(len(call_cnt))]

    nc = bacc.Bacc("TRN2", num_devices=N_CORES, num_swdge_queues=4,
                   dynamic_dma_scratch_size=65536)

    t_idx = nc.dram_tensor("IDX", [128, totslot // 16], i16, kind="ExternalInput")
    t_dstloc = nc.dram_tensor("DSTLOC", [128, T], bf16, kind="ExternalInput")
    t_wcol = nc.dram_tensor("WCOL", [128, T], bf16, kind="ExternalInput")
    t_selfw = nc.dram_tensor("SELFWB", [128, per_pad], bf16, kind="ExternalInput")
    t_xt = nc.dram_tensor("XT", [128, per_pad], f32, kind="ExternalInput")
    t_W = [
        nc.dram_tensor(f"W{i}", [128, 128], f32, kind="ExternalInput") for i in (1, 2, 3)
    ]
    t_b = [
        nc.dram_tensor(f"b{i}", [128, 1], f32, kind="ExternalInput") for i in (1, 2, 3)
    ]
    t_iota_b = nc.dram_tensor("IOTAB", [128, 128], bf16, kind="ExternalInput")
    t_ident = nc.dram_tensor("IDENT", [128, 128], f32, kind="ExternalInput")
    t_identb = nc.dram_tensor("IDENTB", [128, 128], bf16, kind="ExternalInput")
    t_out = nc.dram_tensor("OUT", [per, 128], f32, kind="ExternalOutput")

    hcurA = [
        nc.dram_tensor(f"hcurA{l}", [SA, 128], bf16, kind="Internal") for l in range(3)
    ]
    hcurB = [
        nc.dram_tensor(f"hcurB{l}", [SB, 128], bf16, kind="Internal") for l in range(3)
    ]
    hfullA = [
        nc.dram_tensor(
            f"hfullA{l}", [N_CORES * SA, 128], bf16, kind="Internal",
            addr_space="Shared",
        )
        for l in range(3)
    ]
    hfullB = [
        nc.dram_tensor(
            f"hfullB{l}", [N_CORES * SB, 128], bf16, kind="Internal",
            addr_space="Shared",
        )
        for l in range(3)
    ]
    t_swc = nc.dram_tensor("swcache", [totslot, 128], bf16, kind="Internal")
    rg = [list(range(N_CORES))]

    def chunks512(total):
        out = []
        o = 0
        while o < total:
            w = min(512, total - o)
            out.append((o, w))
            o += w
        return out

    with TileContext(nc) as tc:
        with (
            tc.tile_pool(name="persist", bufs=1) as pp,
            tc.tile_pool(name="work", bufs=2) as wp,
            tc.tile_pool(name="swp", bufs=6) as swp,
            tc.tile_pool(name="psum", bufs=2, space="PSUM") as psp,
            tc.tile_pool(name="psumg", bufs=3, space="PSUM") as pspg,
        ):
            # ---------- persistent loads ----------
            DSTLOC = pp.tile([128, T], bf16, tag="DSTLOC")
            nc.sync.dma_start(DSTLOC[:, :], t_dstloc[:, :])
            WCOL = pp.tile([128, T], bf16, tag="WCOL")
            nc.sync.dma_start(WCOL[:, :], t_wcol[:, :])
            SELFWB = pp.tile([128, per_pad], bf16, tag="SELFWB")
            nc.sync.dma_start(SELFWB[:, :], t_selfw[:, :])
            IOTAB = pp.tile([128, 128], bf16, tag="IOTAB")
            nc.sync.dma_start(IOTAB[:, :], t_iota_b[:, :])
            IDENT = pp.tile([128, 128], f32, tag="IDENT")
            nc.sync.dma_start(IDENT[:, :], t_ident[:, :])
            IDENTB = pp.tile([128, 128], bf16, tag="IDENTB")
            nc.sync.dma_start(IDENTB[:, :], t_identb[:, :])
            W = []
            B = []
            for i in range(3):
                Wt = pp.tile([128, 128], f32, tag=f"W{i}")
                nc.sync.dma_start(Wt[:, :], t_W[i][:, :])
                W.append(Wt)
                Bt = pp.tile([128, 1], f32, tag=f"B{i}")
                nc.sync.dma_start(Bt[:, :], t_b[i][:, :])
                B.append(Bt)

            HOUT = pp.tile([128, per_pad], f32, tag="HOUT")
            HP = pp.tile([128, per_pad], bf16, tag="HP")

            # ---------- h1' = x @ W1 ----------
            for o, cw in chunks512(per_pad):
                xc = wp.tile([128, 512], f32, tag="xc")
                nc.sync.dma_start(xc[:, :cw], t_xt[:, o : o + cw])
                ph = psp.tile([128, 512], f32, tag="p512")
                nc.tensor.matmul(ph[:, :cw], W[0][:, :], xc[:, :cw], start=True, stop=True)
                nc.vector.tensor_copy(HP[:, o : o + cw], ph[:, :cw])

            # ---------- layers ----------
            for l in range(3):
                # HP (feat x node, bf16) -> rows (PE transpose) -> hcurA/B
                # -> two AllGathers (A fires as soon as its rows are stored)
                ag_a_done = False
                for cb in range(nb):
                    pt = psp.tile([128, 128], bf16, tag="ptr", bufs=1)
                    nc.tensor.transpose(
                        pt[:, :], HP[:, cb * 128 : (cb + 1) * 128], IDENTB[:, :]
                    )
                    rt = wp.tile([128, 128], bf16, tag="rowb")
                    nc.vector.tensor_copy(rt[:, :], pt[:, :])
                    r0 = cb * 128
                    r1 = min(per, r0 + 128)
                    if r1 <= SA:
                        nc.sync.dma_start(hcurA[l][r0:r1, :], rt[0 : r1 - r0, :])
                    elif r0 >= SA:
                        nc.sync.dma_start(
                            hcurB[l][r0 - SA : r1 - SA, :], rt[0 : r1 - r0, :]
                        )
                    else:
                        nc.sync.dma_start(hcurA[l][r0:SA, :], rt[0 : SA - r0, :])
                        nc.sync.dma_start(
                            hcurB[l][0 : r1 - SA, :], rt[SA - r0 : r1 - r0, :]
                        )
                    if r1 >= SA and not ag_a_done:
                        nc.gpsimd.collective_compute(
                            "AllGather", mybir.AluOpType.bypass,
                            ins=[hcurA[l][:, :]], outs=[hfullA[l][:, :]],
                            replica_groups=rg,
                        )
                        ag_a_done = True
                nc.gpsimd.collective_compute(
                    "AllGather", mybir.AluOpType.bypass,
                    ins=[hcurB[l][:, :]], outs=[hfullB[l][:, :]],
                    replica_groups=rg,
                )

                for gi, g in enumerate(groups):
                    gw = len(g) * 128
                    mts = {}
                    for h in (0, 1):
                        ci = 2 * gi + h
                        cnt = call_cnt[ci]
                        woff = call_base[ci] // 16
                        idxt = wp.tile([128, maxw16], i16, tag="idx", bufs=24)
                        nc.sync.dma_start(
                            idxt[:, : cnt // 16], t_idx[:, woff : woff + cnt // 16]
                        )
                        mt = wp.tile([128, maxw128, 128], bf16, tag="mtile",
                                     bufs=MT_BUFS)
                        src_tab = hfullA[l][:, :] if h == 0 else hfullB[l][:, :]
                        nc.gpsimd.dma_gather(
                            mt[:, : cnt // 128, :], src_tab, idxt[:, : cnt // 16],
                            num_idxs=cnt, num_idxs_reg=cnt, elem_size=128,
                            single_packet=False, queue_num=ci % 4,
                        )
                        mts[h] = mt
                    sws = {}
                    for h in (0, 1):
                        ci2 = 2 * gi + h
                        cnt2 = call_cnt[ci2]
                        nt2 = cnt2 // 128
                        base2 = call_base[ci2] // 128
                        swl = swp.tile([128, maxw128, 128], bf16, tag="swg")
                        if l == 0:
                            # build S_w on-chip: (dstloc==iota) * norm
                            dl_b = DSTLOC[:, base2 : base2 + nt2].unsqueeze(
                                2
                            ).to_broadcast([128, nt2, 128])
                            io_b = IOTAB[:, :].unsqueeze(1).to_broadcast(
                                [128, nt2, 128]
                            )
                            nc.vector.tensor_tensor(
                                swl[:, :nt2, :], dl_b, io_b,
                                op=mybir.AluOpType.is_equal,
                            )
                            wc_b = WCOL[:, base2 : base2 + nt2].unsqueeze(
                                2
                            ).to_broadcast([128, nt2, 128])
                            nc.vector.tensor_tensor(
                                swl[:, :nt2, :], swl[:, :nt2, :], wc_b,
                                op=mybir.AluOpType.mult,
                            )
                            swv = t_swc[
                                call_base[ci2] : call_base[ci2] + cnt2, :
                            ].rearrange("(t e) d -> e t d", e=128)
                            nc.sync.dma_start(swv, swl[:, :nt2, :])
                        else:
                            swv = t_swc[
                                call_base[ci2] : call_base[ci2] + cnt2, :
                            ].rearrange("(t e) d -> e t d", e=128)
                            nc.sync.dma_start(swl[:, :nt2, :], swv)
                        sws[h] = swl

                    pg = pspg.tile([128, 512], f32, tag="pblk")
                    for bi, b in enumerate(g):
                        cols = tcols_b[b]
                        nlo = int(tiles_bh[b][0])
                        for ti, tcol in enumerate(cols):
                            hh = 0 if ti < nlo else 1
                            j = tcol - call_base[2 * gi + hh] // 128
                            nc.tensor.matmul(
                                pg[:, bi * 128 : (bi + 1) * 128],
                                mts[hh][:, j, :], sws[hh][:, j, :],
                                start=(ti == 0), stop=(ti == len(cols) - 1),
                            )
                    # batched epilogue for the whole group:
                    # out = pg + selfw*h' (+bias, leaky via Prelu)
                    g0 = g[0] * 128
                    ep = wp.tile([128, 512], f32, tag="ep")
                    nc.vector.tensor_tensor(
                        ep[:, :gw], SELFWB[:, g0 : g0 + gw], HP[:, g0 : g0 + gw],
                        op=mybir.AluOpType.mult,
                    )
                    nc.vector.tensor_tensor(
                        ep[:, :gw], ep[:, :gw], pg[:, :gw],
                        op=mybir.AluOpType.add,
                    )
                    if l < 2 and LEAKY_VIA_PRELU:
                        nc.scalar.activation(
                            HOUT[:, g0 : g0 + gw], ep[:, :gw],
                            mybir.ActivationFunctionType.Prelu,
                            bias=B[l][:, 0:1], scale=1.0, alpha=NEG_SLOPE,
                        )
                    elif l < 2:
                        t2 = wp.tile([128, 512], f32, tag="ep2")
                        nc.scalar.activation(
                            t2[:, :gw], ep[:, :gw],
                            mybir.ActivationFunctionType.Identity,
                            bias=B[l][:, 0:1], scale=1.0,
                        )
                        t3 = wp.tile([128, 512], f32, tag="ep3")
                        nc.vector.tensor_scalar_mul(t3[:, :gw], t2[:, :gw], NEG_SLOPE)
                        nc.vector.tensor_tensor(
                            HOUT[:, g0 : g0 + gw], t2[:, :gw], t3[:, :gw],
                            op=mybir.AluOpType.max,
                        )
                    else:
                        nc.scalar.activation(
                            HOUT[:, g0 : g0 + gw], ep[:, :gw],
                            mybir.ActivationFunctionType.Identity,
                            bias=B[l][:, 0:1], scale=1.0,
                        )

                if l < 2:
                    # HP = HOUT @ W[l+1]
                    for o, cw in chunks512(per_pad):
                        ph = psp.tile([128, 512], f32, tag="p512")
                        nc.tensor.matmul(
                            ph[:, :cw], W[l + 1][:, :], HOUT[:, o : o + cw],
                            start=True, stop=True,
                        )
                        nc.vector.tensor_copy(HP[:, o : o + cw], ph[:, :cw])
                else:
                    # final: transpose HOUT (f32) to rows and store
                    for cb in range(nb):
                        pt = psp.tile([128, 128], f32, tag="ptrf", bufs=1)
                        nc.tensor.transpose(
                            pt[:, :], HOUT[:, cb * 128 : (cb + 1) * 128], IDENT[:, :]
                        )
                        rf = wp.tile([128, 128], f32, tag="rowf")
                        nc.vector.tensor_copy(rf[:, :], pt[:, :])
                        r0 = cb * 128
                        r1 = min(per, r0 + 128)
                        nc.sync.dma_start(t_out[r0:r1, :], rf[0 : r1 - r0, :])

    nc.compile()
    return nc


_CACHE = {}


def kernel(
    x,
    edge_index,
    edge_attr,
    edge_type,
    edge_type_scale,
    W1,
    b1,
    W2,
    b2,
    W3,
    b3,
):
    x = np.asarray(x)
    Nn = x.shape[0]
    meta, per_core = _preprocess(
        np.asarray(x), np.asarray(edge_index), np.asarray(edge_attr),
        np.asarray(edge_type), np.asarray(edge_type_scale),
    )

    key = (Nn, meta["T"], tuple(meta["call_cnt"]))
    if key not in _CACHE:
        _CACHE[key] = _build(meta)
    nc = _CACHE[key]

    iota_f = np.tile(np.arange(128, dtype=np.float32)[None, :], (128, 1))
    ident = np.eye(128, dtype=np.float32)
    common = dict(
        W1=np.asarray(W1, np.float32),
        W2=np.asarray(W2, np.float32),
        W3=np.asarray(W3, np.float32),
        b1=np.asarray(b1, np.float32).reshape(D, 1),
        b2=np.asarray(b2, np.float32).reshape(D, 1),
        b3=np.asarray(b3, np.float32).reshape(D, 1),
        IOTAB=iota_f.astype(BF16),
        IDENT=ident,
        IDENTB=ident.astype(BF16),
    )
    in_maps = []
    for c in range(N_CORES):
        m = dict(common)
        m.update(per_core[c])
        in_maps.append(m)

    res = run_bass_kernel_spmd(
        nc, in_maps, core_ids=list(range(N_CORES)), **_RUN_KWARGS
    )
    _LAST_RESULT.clear()
    _LAST_RESULT["exec_time_ns"] = res.exec_time_ns
    _LAST_RESULT["profile_json"] = res.profile_json
    out = np.concatenate([res.results[c]["OUT"] for c in range(N_CORES)], axis=0)
    return out.astype(np.float32)


_RUN_KWARGS = {}  # test harness can set {"trace": True, "tmpdir": ...}
_LAST_RESULT = {}


# revision 13
# speedup vs baseline: 1.6480x; 1.1567x over previous
"""3-layer GCN (message passing) on 8 TRN2 NeuronCores.

Strategy: shard destination nodes across cores (graph parallel). Each layer:
  h'_T = prev @ W  computed locally on the node shard (PE),
  AllGather h' rows (bf16) so every core sees all source features,
  per (dst block, lo/hi half): dma_gather source rows on 4 parallel SWDGE
  queues (edges sorted by dst block, lo/hi split so gather indices fit
  int16; pad slots gather row 0 with weight 0),
  S_w[e,d] = (dstloc[e]==d)*norm[e] built ON-CHIP by DVE in layer 1 and
  cached in DRAM for layers 2-3, and out_T = M.T @ S_w accumulates on the
  PE into a group-wide PSUM bank. Epilogue: out = pg + selfw*h' + bias,
  LeakyReLU. GCN symmetric normalization (including self loops) is folded
  into per-edge weights (WCOL) and self weights (SELFWB) on the host, so
  there is no on-device degree pass.
"""

import numpy as np

import concourse.bacc as bacc
import concourse.mybir as mybir
from concourse.tile import TileContext
from concourse.bass_utils import run_bass_kernel_spmd

try:
    import ml_dtypes

    BF16 = ml_dtypes.bfloat16
except ImportError:  # pragma: no cover
    BF16 = None

N_CORES = 8
D = 128
NEG_SLOPE = 0.1
G_BLOCKS = 1  # dst blocks per gather call group
LEAKY_VIA_PRELU = True
MT_BUFS = 12


def _ceil_div(a, b):
    return (a + b - 1) // b


def _wrap_idx(idx):
    """[cnt] int16 -> [128, cnt//16] wrapped layout (16-partition, replicated x8)."""
    cnt = idx.shape[0]
    assert cnt % 16 == 0
    w = idx.reshape(cnt // 16, 16).T  # [16, cnt//16]
    return np.tile(w, (8, 1)).astype(np.int16)  # [128, cnt//16]


def _preprocess(x, edge_index, edge_attr, edge_type, edge_type_scale):
    """Host-side sharding/layout. Returns (meta, per-core input arrays)."""
    Nn = x.shape[0]
    assert Nn % N_CORES == 0
    per = Nn // N_CORES
    nb = _ceil_div(per, 128)
    per_pad = nb * 128
    # split each core's shard rows at SA: half A rows [0,SA), half B [SA,per).
    SA = max(16, ((per // 2) // 16) * 16)
    SB = per - SA
    assert SA * N_CORES <= 32767 + 1 and SB * N_CORES <= 32767 + 1

    src_f = np.asarray(edge_index[0], dtype=np.int64)
    dst_f = np.asarray(edge_index[1], dtype=np.int64)
    w = np.asarray(edge_type_scale, np.float32)[
        np.asarray(edge_type, np.int64)
    ] * np.asarray(edge_attr, np.float32)
    # symmetric GCN norm with self loops, computed on host
    deg = np.bincount(dst_f, weights=w, minlength=Nn).astype(np.float32) + 1.0
    dinv = 1.0 / np.sqrt(deg)
    norm = (dinv[src_f] * w * dinv[dst_f]).astype(np.float32)
    selfw = (dinv * dinv).astype(np.float32)

    core = dst_f // per
    ldst = dst_f - core * per
    blk = ldst >> 7
    slot = ldst & 127
    src_c = src_f // per
    src_r = src_f - src_c * per
    half = (src_r >= SA).astype(np.int64)
    gidx = np.where(half == 0, src_c * SA + src_r, src_c * SB + (src_r - SA))

    counts = np.zeros((N_CORES, nb, 2), dtype=np.int64)
    per_core = []
    for c in range(N_CORES):
        m = core == c
        s_src = src_f[m]
        s_blk = blk[m]
        s_half = half[m]
        order = np.lexsort((s_src, s_half, s_blk))
        per_core.append(
            dict(
                src=gidx[m][order],
                half=s_half[order],
                blk=s_blk[order],
                slot=slot[m][order],
                norm=norm[m][order],
            )
        )
        cnt = np.bincount(s_blk * 2 + s_half, minlength=nb * 2).reshape(nb, 2)
        counts[c] = cnt

    # common padded schedule: tiles per (block, half), maxed over cores
    tiles_bh = np.maximum(1, _ceil_div(counts.max(axis=0), 128))  # [nb, 2]
    pad_bh = tiles_bh * 128

    groups = [list(range(g, min(g + G_BLOCKS, nb))) for g in range(0, nb, G_BLOCKS)]
    slot_off = np.zeros((nb, 2), dtype=np.int64)
    call_cnt = []  # per (g, half): total padded count
    off = 0
    for g in groups:
        for h in (0, 1):
            c0 = off
            for b in g:
                slot_off[b, h] = off
                off += pad_bh[b, h]
            call_cnt.append(off - c0)
    totslot = off
    T = totslot // 128

    tcols_b = []
    for b in range(nb):
        cols = list(range(slot_off[b, 0] // 128, slot_off[b, 0] // 128 + tiles_bh[b, 0]))
        cols += list(range(slot_off[b, 1] // 128, slot_off[b, 1] // 128 + tiles_bh[b, 1]))
        tcols_b.append(cols)

    ins = []
    for c in range(N_CORES):
        pc = per_core[c]
        idx_sl = np.zeros(totslot, dtype=np.int16)  # pads gather row 0 (norm=0)
        dst_sl = np.zeros(totslot, dtype=np.float32)
        wc_sl = np.zeros(totslot, dtype=np.float32)
        e0 = 0
        for b in range(nb):
            for h in (0, 1):
                n = counts[c, b, h]
                o = slot_off[b, h]
                if n:
                    sl = slice(e0, e0 + n)
                    idx_sl[o : o + n] = pc["src"][sl].astype(np.int16)
                    dst_sl[o : o + n] = pc["slot"][sl]
                    wc_sl[o : o + n] = pc["norm"][sl]
                    e0 += n

        wrapped = []
        off2 = 0
        for cc in call_cnt:
            wrapped.append(_wrap_idx(idx_sl[off2 : off2 + cc]))
            off2 += cc
        idx_w = np.concatenate(wrapped, axis=1)  # [128, totslot//16]

        col = lambda a: np.ascontiguousarray(a.reshape(T, 128).T)  # [128, T]
        xt = np.zeros((128, per_pad), dtype=np.float32)
        xt[:, :per] = np.asarray(x[c * per : (c + 1) * per], dtype=np.float32).T
        sw2 = np.zeros((128, per_pad), dtype=np.float32)
        sw2[:, :per] = selfw[c * per : (c + 1) * per][None, :]
        ins.append(
            dict(
                IDX=idx_w,
                DSTLOC=col(dst_sl).astype(BF16),
                WCOL=col(wc_sl).astype(BF16),
                SELFWB=sw2.astype(BF16),
                XT=xt,
            )
        )

    meta = dict(
        N=Nn, per=per, nb=nb, per_pad=per_pad, SA=SA, T=T,
        totslot=totslot, groups=groups, call_cnt=call_cnt, tiles_bh=tiles_bh,
        slot_off=slot_off, tcols_b=tcols_b,
    )
    return meta, ins


def _build(meta):
    per = meta["per"]
    nb = meta["nb"]
    per_pad = meta["per_pad"]
    SA = meta["SA"]
    SB = per - SA
    T = meta["T"]
    totslot = meta["totslot"]
    groups = meta["groups"]
    call_cnt = meta["call_cnt"]
    tiles_bh = meta["tiles_bh"]
    tcols_b = meta["tcols_b"]

    f32 = mybir.dt.float32
    bf16 = mybir.dt.bfloat16
    i16 = mybir.dt.int16

    maxw16 = max(c // 16 for c in call_cnt)
    maxw128 = max(c // 128 for c in call_cnt)
    call_base = [sum(call_cnt[:i]) for i in range(len(call_cnt))]

    nc = bacc.Bacc("TRN2", num_devices=N_CORES, num_swdge_queues=4,
                   dynamic_dma_scratch_size=65536)

    t_idx = nc.dram_tensor("IDX", [128, totslot // 16], i16, kind="ExternalInput")
    t_dstloc = nc.dram_tensor("DSTLOC", [128, T], bf16, kind="ExternalInput")
    t_wcol = nc.dram_tensor("WCOL", [128, T], bf16, kind="ExternalInput")
    t_selfw = nc.dram_tensor("SELFWB", [128, per_pad], bf16, kind="ExternalInput")
    t_xt = nc.dram_tensor("XT", [128, per_pad], f32, kind="ExternalInput")
    t_W = [
        nc.dram_tensor(f"W{i}", [128, 128], f32, kind="ExternalInput") for i in (1, 2, 3)
    ]
    t_b = [
        nc.dram_tensor(f"b{i}", [128, 1], f32, kind="ExternalInput") for i in (1, 2, 3)
    ]
    t_iota_b = nc.dram_tensor("IOTAB", [128, 128], bf16, kind="ExternalInput")
    t_ident = nc.dram_tensor("IDENT", [128, 128], f32, kind="ExternalInput")
    t_identb = nc.dram_tensor("IDENTB", [128, 128], bf16, kind="ExternalInput")
    t_out = nc.dram_tensor("OUT", [per, 128], f32, kind="ExternalOutput")

    hcurA = [
        nc.dram_tensor(f"hcurA{l}", [SA, 128], bf16, kind="Internal") for l in range(3)
    ]
    hcurB = [
        nc.dram_tensor(f"hcurB{l}", [SB, 128], bf16, kind="Internal") for l in range(3)
    ]
    hfullA = [
        nc.dram_tensor(
            f"hfullA{l}", [N_CORES * SA, 128], bf16, kind="Internal",
            addr_space="Shared",
        )
        for l in range(3)
    ]
    hfullB = [
        nc.dram_tensor(
            f"hfullB{l}", [N_CORES * SB, 128], bf16, kind="Internal",
            addr_space="Shared",
        )
        for l in range(3)
    ]
    t_swc = nc.dram_tensor("swcache", [128, T, 128], bf16, kind="Internal")
    rg = [list(range(N_CORES))]

    def chunks512(total):
        out = []
        o = 0
        while o < total:
            w = min(512, total - o)
            out.append((o, w))
            o += w
        return out

    with TileContext(nc) as tc:
        with (
            tc.tile_pool(name="persist", bufs=1) as pp,
            tc.tile_pool(name="work", bufs=2) as wp,
            tc.tile_pool(name="swp", bufs=6) as swp,
            tc.tile_pool(name="psum", bufs=2, space="PSUM") as psp,
            tc.tile_pool(name="psumg", bufs=3, space="PSUM") as pspg,
        ):
            # ---------- persistent loads ----------
            IDXT = pp.tile([128, totslot // 16], i16, tag="IDXT")
            nc.sync.dma_start(IDXT[:, :], t_idx[:, :])
            DSTLOC = pp.tile([128, T], bf16, tag="DSTLOC")
            nc.sync.dma_start(DSTLOC[:, :], t_dstloc[:, :])
            WCOL = pp.tile([128, T], bf16, tag="WCOL")
            nc.sync.dma_start(WCOL[:, :], t_wcol[:, :])
            SELFWB = pp.tile([128, per_pad], bf16, tag="SELFWB")
            nc.sync.dma_start(SELFWB[:, :], t_selfw[:, :])
            IOTAB = pp.tile([128, 128], bf16, tag="IOTAB")
            nc.sync.dma_start(IOTAB[:, :], t_iota_b[:, :])
            IDENT = pp.tile([128, 128], f32, tag="IDENT")
            nc.sync.dma_start(IDENT[:, :], t_ident[:, :])
            IDENTB = pp.tile([128, 128], bf16, tag="IDENTB")
            nc.sync.dma_start(IDENTB[:, :], t_identb[:, :])
            W = []
            B = []
            for i in range(3):
                Wt = pp.tile([128, 128], f32, tag=f"W{i}")
                nc.sync.dma_start(Wt[:, :], t_W[i][:, :])
                W.append(Wt)
                Bt = pp.tile([128, 1], f32, tag=f"B{i}")
                nc.sync.dma_start(Bt[:, :], t_b[i][:, :])
                B.append(Bt)

            HOUT = pp.tile([128, per_pad], f32, tag="HOUT")
            HP = pp.tile([128, per_pad], bf16, tag="HP")

            # ---------- h1' = x @ W1 ----------
            for o, cw in chunks512(per_pad):
                xc = wp.tile([128, 512], f32, tag="xc")
                nc.sync.dma_start(xc[:, :cw], t_xt[:, o : o + cw])
                ph = psp.tile([128, 512], f32, tag="p512")
                nc.tensor.matmul(ph[:, :cw], W[0][:, :], xc[:, :cw], start=True, stop=True)
                nc.vector.tensor_copy(HP[:, o : o + cw], ph[:, :cw])

            # ---------- layers ----------
            for l in range(3):
                # HP (feat x node, bf16) -> rows (PE transpose) -> hcurA/B
                # -> two AllGathers (A fires as soon as its rows are stored)
                ag_a_done = False
                for cb in range(nb):
                    pt = psp.tile([128, 128], bf16, tag="ptr", bufs=1)
                    nc.tensor.transpose(
                        pt[:, :], HP[:, cb * 128 : (cb + 1) * 128], IDENTB[:, :]
                    )
                    rt = wp.tile([128, 128], bf16, tag="rowb")
                    nc.vector.tensor_copy(rt[:, :], pt[:, :])
                    r0 = cb * 128
                    r1 = min(per, r0 + 128)
                    if r1 <= SA:
                        nc.sync.dma_start(hcurA[l][r0:r1, :], rt[0 : r1 - r0, :])
                    elif r0 >= SA:
                        nc.sync.dma_start(
                            hcurB[l][r0 - SA : r1 - SA, :], rt[0 : r1 - r0, :]
                        )
                    else:
                        nc.sync.dma_start(hcurA[l][r0:SA, :], rt[0 : SA - r0, :])
                        nc.sync.dma_start(
                            hcurB[l][0 : r1 - SA, :], rt[SA - r0 : r1 - r0, :]
                        )
                    if r1 >= SA and not ag_a_done:
                        nc.gpsimd.collective_compute(
                            "AllGather", mybir.AluOpType.bypass,
                            ins=[hcurA[l][:, :]], outs=[hfullA[l][:, :]],
                            replica_groups=rg,
                        )
                        ag_a_done = True
                nc.gpsimd.collective_compute(
                    "AllGather", mybir.AluOpType.bypass,
                    ins=[hcurB[l][:, :]], outs=[hfullB[l][:, :]],
                    replica_groups=rg,
                )

                for gi, g in enumerate(groups):
                    gw = len(g) * 128
                    mts = {}
                    for h in (0, 1):
                        ci = 2 * gi + h
                        cnt = call_cnt[ci]
                        woff = call_base[ci] // 16
                        mt = wp.tile([128, maxw128, 128], bf16, tag="mtile",
                                     bufs=MT_BUFS)
                        src_tab = hfullA[l][:, :] if h == 0 else hfullB[l][:, :]
                        nc.gpsimd.dma_gather(
                            mt[:, : cnt // 128, :], src_tab,
                            IDXT[:, woff : woff + cnt // 16],
                            num_idxs=cnt, num_idxs_reg=cnt, elem_size=128,
                            single_packet=False, queue_num=ci % 4,
                        )
                        mts[h] = mt
                    sws = {}
                    for h in (0, 1):
                        ci2 = 2 * gi + h
                        cnt2 = call_cnt[ci2]
                        nt2 = cnt2 // 128
                        base2 = call_base[ci2] // 128
                        swl = swp.tile([128, maxw128, 128], bf16, tag="swg")
                        if l == 0:
                            # build S_w on-chip: (dstloc==iota) * norm
                            dl_b = DSTLOC[:, base2 : base2 + nt2].unsqueeze(
                                2
                            ).to_broadcast([128, nt2, 128])
                            io_b = IOTAB[:, :].unsqueeze(1).to_broadcast(
                                [128, nt2, 128]
                            )
                            nc.vector.tensor_tensor(
                                swl[:, :nt2, :], dl_b, io_b,
                                op=mybir.AluOpType.is_equal,
                            )
                            wc_b = WCOL[:, base2 : base2 + nt2].unsqueeze(
                                2
                            ).to_broadcast([128, nt2, 128])
                            nc.vector.tensor_tensor(
                                swl[:, :nt2, :], swl[:, :nt2, :], wc_b,
                                op=mybir.AluOpType.mult,
                            )
                            nc.sync.dma_start(
                                t_swc[:, base2 : base2 + nt2, :], swl[:, :nt2, :]
                            )
                        else:
                            nc.sync.dma_start(
                                swl[:, :nt2, :], t_swc[:, base2 : base2 + nt2, :]
                            )
                        sws[h] = swl

                    pg = pspg.tile([128, 512], f32, tag="pblk")
                    for bi, b in enumerate(g):
                        cols = tcols_b[b]
                        nlo = int(tiles_bh[b][0])
                        for ti, tcol in enumerate(cols):
                            hh = 0 if ti < nlo else 1
                            j = tcol - call_base[2 * gi + hh] // 128
                            nc.tensor.matmul(
                                pg[:, bi * 128 : (bi + 1) * 128],
                                mts[hh][:, j, :], sws[hh][:, j, :],
                                start=(ti == 0), stop=(ti == len(cols) - 1),
                            )
                    # batched epilogue for the whole group:
                    # out = pg + selfw*h' (+bias, leaky via Prelu)
                    g0 = g[0] * 128
                    ep = wp.tile([128, 512], f32, tag="ep")
                    nc.vector.tensor_tensor(
                        ep[:, :gw], SELFWB[:, g0 : g0 + gw], HP[:, g0 : g0 + gw],
                        op=mybir.AluOpType.mult,
                    )
                    nc.vector.tensor_tensor(
                        ep[:, :gw], ep[:, :gw], pg[:, :gw],
                        op=mybir.AluOpType.add,
                    )
                    if l < 2 and LEAKY_VIA_PRELU:
                        nc.scalar.activation(
                            HOUT[:, g0 : g0 + gw], ep[:, :gw],
                            mybir.ActivationFunctionType.Prelu,
                            bias=B[l][:, 0:1], scale=1.0, alpha=NEG_SLOPE,
                        )
                    elif l < 2:
                        t2 = wp.tile([128, 512], f32, tag="ep2")
                        nc.scalar.activation(
                            t2[:, :gw], ep[:, :gw],
                            mybir.ActivationFunctionType.Identity,
                            bias=B[l][:, 0:1], scale=1.0,
                        )
                        t3 = wp.tile([128, 512], f32, tag="ep3")
                        nc.vector.tensor_scalar_mul(t3[:, :gw], t2[:, :gw], NEG_SLOPE)
                        nc.vector.tensor_tensor(
                            HOUT[:, g0 : g0 + gw], t2[:, :gw], t3[:, :gw],
                            op=mybir.AluOpType.max,
                        )
                    else:
                        nc.scalar.activation(
                            HOUT[:, g0 : g0 + gw], ep[:, :gw],
                            mybir.ActivationFunctionType.Identity,
                            bias=B[l][:, 0:1], scale=1.0,
                        )

                if l < 2:
                    # HP = HOUT @ W[l+1]
                    for o, cw in chunks512(per_pad):
                        ph = psp.tile([128, 512], f32, tag="p512")
                        nc.tensor.matmul(
                            ph[:, :cw], W[l + 1][:, :], HOUT[:, o : o + cw],
                            start=True, stop=True,
                        )
                        nc.vector.tensor_copy(HP[:, o : o + cw], ph[:, :cw])
                else:
                    # final: transpose HOUT (f32) to rows and store
                    for cb in range(nb):
                        pt = psp.tile([128, 128], f32, tag="ptrf", bufs=1)
                        nc.tensor.transpose(
                            pt[:, :], HOUT[:, cb * 128 : (cb + 1) * 128], IDENT[:, :]
                        )
                        rf = wp.tile([128, 128], f32, tag="rowf")
                        nc.vector.tensor_copy(rf[:, :], pt[:, :])
                        r0 = cb * 128
                        r1 = min(per, r0 + 128)
                        nc.sync.dma_start(t_out[r0:r1, :], rf[0 : r1 - r0, :])

    nc.compile()
    return nc


_CACHE = {}


def kernel(
    x,
    edge_index,
    edge_attr,
    edge_type,
    edge_type_scale,
    W1,
    b1,
    W2,
    b2,
    W3,
    b3,
):
    x = np.asarray(x)
    Nn = x.shape[0]
    meta, per_core = _preprocess(
        np.asarray(x), np.asarray(edge_index), np.asarray(edge_attr),
        np.asarray(edge_type), np.asarray(edge_type_scale),
    )

    key = (Nn, meta["T"], tuple(meta["call_cnt"]))
    if key not in _CACHE:
        _CACHE[key] = _build(meta)
    nc = _CACHE[key]

    iota_f = np.tile(np.arange(128, dtype=np.float32)[None, :], (128, 1))
    ident = np.eye(128, dtype=np.float32)
    common = dict(
        W1=np.asarray(W1, np.float32),
        W2=np.asarray(W2, np.float32),
        W3=np.asarray(W3, np.float32),
        b1=np.asarray(b1, np.float32).reshape(D, 1),
        b2=np.asarray(b2, np.float32).reshape(D, 1),
        b3=np.asarray(b3, np.float32).reshape(D, 1),
        IOTAB=iota_f.astype(BF16),
        IDENT=ident,
        IDENTB=ident.astype(BF16),
    )
    in_maps = []
    for c in range(N_CORES):
        m = dict(common)
        m.update(per_core[c])
        in_maps.append(m)

    res = run_bass_kernel_spmd(
        nc, in_maps, core_ids=list(range(N_CORES)), **_RUN_KWARGS
    )
    _LAST_RESULT.clear()
    _LAST_RESULT["exec_time_ns"] = res.exec_time_ns
    _LAST_RESULT["profile_json"] = res.profile_json
    out = np.concatenate([res.results[c]["OUT"] for c in range(N_CORES)], axis=0)
    return out.astype(np.float32)


_RUN_KWARGS = {}  # test harness can set {"trace": True, "tmpdir": ...}
_LAST_RESULT = {}
